# revision 4
# baseline (speedup 1.0000x reference)
"""MaxUnpooling2D scatter-add kernel for Trainium2 (8 NeuronCores).

Problem: updates/mask [32,112,112,64] f32/int32 -> out [32,224,224,64] f32,
out[b, y, x, c] += updates[b, h, w, c] with y/x decoded from mask (random
full-range indices, duplicates summed).

Strategy (all scatter work on device):
  - Shard by batch-pair: 16 pairs x 2 column-halves = 32 work units over
    8 cores x 4 sequential invocations of ONE compiled module.
  - Data laid out plane-major ([batch-local, channel] -> partition) so each
    indirect-DMA call scatters 128 elements from 128 DISJOINT output planes:
    offsets within a call are structurally unique (no duplicate races).
  - Scatter via gpsimd indirect DMA with CCE accumulate (compute_op=add);
    calls on the same SWDGE queue serialize, so cross-call duplicate indices
    accumulate exactly (verified on hardware).
  - ExternalOutput buffers are pre-zeroed by the runtime; partial outputs of
    the two column-halves of a pair are summed on host.
"""
import numpy as np

import concourse.bacc as bacc
import concourse.bass as bass
import concourse.mybir as mybir
import concourse.tile as tile
from concourse.bass_utils import run_bass_kernel_spmd

B, H, W, C = 32, 112, 112, 64
OUT_HW = (2 * H) * (2 * W)            # 224*224
PLANE = OUT_HW                         # bins per (b, c) plane = 50176
BATCH_BINS = OUT_HW * C                # 3211264
PAIR_BINS = 2 * BATCH_BINS             # 6422528
NPOS = H * W                           # 12544 source positions per batch
HALF = NPOS // 2                       # 6272 columns per module invocation
N_CORES = 8

_cached_nc = None


def _build_module():
    """One core's work unit: scatter [128 planes x HALF cols] into a
    2-batch output with exact duplicate accumulation."""
    nc = bacc.Bacc("TRN2", target_bir_lowering=False, debug=False)
    val_d = nc.dram_tensor("val", [128, HALF], mybir.dt.float32, kind="ExternalInput")
    msk_d = nc.dram_tensor("msk", [128, HALF], mybir.dt.int32, kind="ExternalInput")
    basec_d = nc.dram_tensor("basec", [128, 1], mybir.dt.int32, kind="ExternalInput")
    bbase_d = nc.dram_tensor("bbase", [128, 1], mybir.dt.int32, kind="ExternalInput")
    neg64_d = nc.dram_tensor("neg64", [128, 1], mybir.dt.int32, kind="ExternalInput")
    out_d = nc.dram_tensor("out", [PAIR_BINS, 1], mybir.dt.float32, kind="ExternalOutput")

    with tile.TileContext(nc) as tc:
        with tc.tile_pool(name="sbuf", bufs=1) as pool:
            val = pool.tile([128, HALF], mybir.dt.float32)
            msk = pool.tile([128, HALF], mybir.dt.int32)
            off = pool.tile([128, HALF], mybir.dt.int32)
            basec = pool.tile([128, 1], mybir.dt.int32)
            bbase = pool.tile([128, 1], mybir.dt.int32)
            neg64 = pool.tile([128, 1], mybir.dt.int32)
            nc.sync.dma_start(out=val[:], in_=val_d[:])
            nc.sync.dma_start(out=msk[:], in_=msk_d[:])
            nc.sync.dma_start(out=basec[:], in_=basec_d[:])
            nc.sync.dma_start(out=bbase[:], in_=bbase_d[:])
            nc.sync.dma_start(out=neg64[:], in_=neg64_d[:])
            # off = ((msk & -64) | c) + batch_local*BATCH_BINS
            # low 6 bits of (msk & -64) are zero, c < 64 -> OR == ADD there
            nc.vector.scalar_tensor_tensor(
                out=off[:],
                in0=msk[:],
                scalar=neg64[:, 0:1],
                in1=basec[:, 0:1].to_broadcast([128, HALF]),
                op0=mybir.AluOpType.bitwise_and,
                op1=mybir.AluOpType.bitwise_or,
            )
            nc.vector.tensor_tensor(
                out=off[:],
                in0=off[:],
                in1=bbase[:, 0:1].to_broadcast([128, HALF]),
                op=mybir.AluOpType.add,
            )
            for j in range(HALF):
                nc.gpsimd.indirect_dma_start(
                    out=out_d[:],
                    out_offset=bass.IndirectOffsetOnAxis(ap=off[:, j:j + 1], axis=0),
                    in_=val[:, j:j + 1],
                    in_offset=None,
                    compute_op=mybir.AluOpType.add,
                )
    nc.compile()
    return nc


def _get_module():
    global _cached_nc
    if _cached_nc is None:
        _cached_nc = _build_module()
    return _cached_nc


def kernel(updates: np.ndarray, mask: np.ndarray) -> np.ndarray:
    assert updates.shape == (B, H, W, C) and mask.shape == (B, H, W, C)
    updates = np.ascontiguousarray(updates, dtype=np.float32)
    mask = np.ascontiguousarray(mask, dtype=np.int32)

    # plane-major: [B, C, NPOS]
    upd_t = np.ascontiguousarray(updates.reshape(B, NPOS, C).transpose(0, 2, 1))
    msk_t = np.ascontiguousarray(mask.reshape(B, NPOS, C).transpose(0, 2, 1))

    # per-partition constants: partition p = (batch_local = p//64, c = p%64)
    p = np.arange(128, dtype=np.int32)
    basec = (p % C).reshape(128, 1).astype(np.int32)
    bbase = ((p // C) * BATCH_BINS).reshape(128, 1).astype(np.int32)
    neg64 = np.full((128, 1), -64, dtype=np.int32)

    nc = _get_module()

    # 32 work units: (pair bp in 0..16) x (half h in 0..2)
    # core = bp % 8, invocation = (bp // 8) * 2 + h
    partials = {}  # (bp, h) -> [PAIR_BINS] f32
    for inv in range(4):
        grp, h = divmod(inv, 2)
        in_maps = []
        for core in range(N_CORES):
            bp = grp * 8 + core
            b0 = 2 * bp
            val = upd_t[b0:b0 + 2, :, h * HALF:(h + 1) * HALF].reshape(128, HALF)
            msk = msk_t[b0:b0 + 2, :, h * HALF:(h + 1) * HALF].reshape(128, HALF)
            in_maps.append({
                "val": np.ascontiguousarray(val),
                "msk": np.ascontiguousarray(msk),
                "basec": basec,
                "bbase": bbase,
                "neg64": neg64,
            })
        res = run_bass_kernel_spmd(nc, in_maps, core_ids=list(range(N_CORES)))
        for core in range(N_CORES):
            bp = grp * 8 + core
            partials[(bp, h)] = res.results[core]["out"].ravel()

    out = np.empty((B, 2 * H, 2 * W, C), dtype=np.float32)
    for bp in range(16):
        tot = partials[(bp, 0)] + partials[(bp, 1)]
        out[2 * bp:2 * bp + 2] = tot.reshape(2, 2 * H, 2 * W, C)
    return out
